# revision 7
# baseline (speedup 1.0000x reference)
"""Trainium2 Bass kernel for nn_HetAttn (heterogeneous-relation GNN).

Computation (see reference):
    h0 = tanh(x @ feat_w)
    het_layer(h, l): per relation r: GraphConv(h) with symmetric norm +
        residual, then softmax-attention over the 3 relations.
    h1 = het(h0, 1); t = het(h0, 0); h2, attn = het(t, 1)
    y = concat([h0,h1,h2]) @ concat_w.T ;  returns (y, attn.T)

Distribution over 8 NeuronCores (graph/data parallel by destination node
range): nodes padded to 50176 = 8*6272; core c owns destination rows
[c*6272, (c+1)*6272).  Edges are bucketed by destination window (128
nodes) on the host; the per-edge symmetric normalization
rsqrt(deg_out[src])*rsqrt(deg_in[dst]) is folded into a per-edge weight.
On device, each core:
  - computes the full (replicated) h0 and writes it to its HBM,
  - per (relation, window): gathers h[src] rows with the Q7 dma_gather
    (block of 128 edges -> 128 partitions), builds a weighted one-hot
    selection matrix on the vector engine (iota == dst_offset) * w, and
    aggregates with PE matmuls into PSUM (transposed acc [feat, node]),
  - applies conv weight / bias / residual, relation attention, and the
    final concat projection,
  - one AllGather moves the middle layer t to every core (overlapped with
    the independent h1 layer).
dma_gather indices are int16, so gathers are split into low (src < 32768,
base h[0:]) and high (src >= 32768, base h[32768:]) streams.
Weight tensors are small and replicated.

kernel(**inputs) takes the FULL inputs and returns the FULL outputs
(y [50000,128] f32, attn [50000,3] f32) matching reference().
"""

import sys
import types
from dataclasses import dataclass

if "/opt/trn_rl_repo" not in sys.path:
    sys.path.insert(0, "/opt/trn_rl_repo")

import numpy as np

import concourse.bacc as bacc
import concourse.bass as bass
import concourse.mybir as mybir
import concourse.tile as tile
from concourse.bass import IndirectOffsetOnAxis
from concourse.bass_utils import run_bass_kernel_spmd

F32 = mybir.dt.float32
BF16 = mybir.dt.bfloat16
I32 = mybir.dt.int32
I16 = mybir.dt.int16
ALU = mybir.AluOpType
ACTF = mybir.ActivationFunctionType


@dataclass(frozen=True)
class Cfg:
    N: int = 50000          # real nodes
    NPAD: int = 50176       # padded to C * W * 128
    C: int = 8              # cores
    D: int = 128            # feature dim
    Q: int = 64             # attention query dim
    R: int = 3              # relations
    KT: int = 8             # x tiles per load in the h0 phase
    G_W: int = 4            # dest windows per processing group
    SUBCAP: int = 64        # max blocks (of 128 edges) per gather instr
    HI0: int = 32768        # int16 gather index split point
    gather_bf16: bool = False   # store h in HBM as bf16 + bf16 aggregation

    @property
    def NP(self):  # nodes per core
        return self.NPAD // self.C

    @property
    def W(self):   # 128-node windows per core
        return self.NP // 128

    @property
    def NT(self):  # 128-row tiles over all nodes
        return self.NPAD // 128

    @property
    def gdt(self):
        return BF16 if self.gather_bf16 else F32


FULL_CFG = Cfg()


def make_groups(cfg: Cfg):
    gs, w0 = [], 0
    while w0 < cfg.W:
        nw = min(cfg.G_W, cfg.W - w0)
        gs.append((w0, nw))
        w0 += nw
    return gs


def split_cols(c0: int, c1: int, cap: int):
    n = c1 - c0
    if n == 0:
        return []
    npc = -(-n // cap)
    cuts = np.linspace(c0, c1, npc + 1).astype(int)
    return [(int(cuts[i]), int(cuts[i + 1])) for i in range(npc)]


# ----------------------------------------------------------------------------
# Host-side preprocessing: shard + sort edges, fold degree normalization.
# ----------------------------------------------------------------------------

class RelMeta:
    """Block-column layout for one relation (shared by all cores).

    Columns are grouped per window-group g: first the low-half (src < HI0)
    blocks of the group's windows in order, then the high-half blocks.
    """

    def __init__(self, Blo, Bhi, groups):
        W = len(Blo)
        self.Blo, self.Bhi = Blo, Bhi
        self.lo_start = np.zeros(W, np.int64)
        self.hi_start = np.zeros(W, np.int64)
        self.g_span = []          # (col0, n_lo, n_hi) per group
        col = 0
        for (w0, nw) in groups:
            g0 = col
            for wl in range(w0, w0 + nw):
                self.lo_start[wl] = col
                col += Blo[wl]
            nlo = col - g0
            for wl in range(w0, w0 + nw):
                self.hi_start[wl] = col
                col += Bhi[wl]
            self.g_span.append((g0, nlo, col - g0 - nlo))
        self.NB = col

    def win_cols(self, wl):
        lo = list(range(self.lo_start[wl], self.lo_start[wl] + self.Blo[wl]))
        hi = list(range(self.hi_start[wl], self.hi_start[wl] + self.Bhi[wl]))
        return lo + hi


def preprocess_edges(edges: np.ndarray, cfg: Cfg):
    """Per relation: (idx16 [C, 128, 8*NB] i16, eoff [C, 128, NB] f32,
    ew [C, 128, NB] f32, RelMeta).  idx16 is the dma_gather index table
    (wrapped in 16 partitions, replicated 8x); high-half columns hold
    src - HI0."""
    N, C, W, HI0 = cfg.N, cfg.C, cfg.W, cfg.HI0
    NW = C * W
    groups = make_groups(cfg)
    out = []
    for r in range(cfg.R):
        src = np.asarray(edges[r, 0], dtype=np.int64)
        dst = np.asarray(edges[r, 1], dtype=np.int64)
        deg_out = np.bincount(src, minlength=N).clip(1).astype(np.float64)
        deg_in = np.bincount(dst, minlength=N).clip(1).astype(np.float64)
        w = ((1.0 / np.sqrt(deg_out))[src] * (1.0 / np.sqrt(deg_in))[dst])

        key = (dst >> 7) * 2 + (src >= HI0)
        order = np.argsort(key, kind="stable")
        src_s, dst_s, w_s, key_s = src[order], dst[order], w[order], key[order]
        cnt = np.bincount(key_s, minlength=NW * 2)          # [NW*2]
        percw = cnt.reshape(C, W, 2)
        Blo = np.ceil(percw[:, :, 0] / 128.0).astype(np.int64).max(axis=0)
        Bhi = np.ceil(percw[:, :, 1] / 128.0).astype(np.int64).max(axis=0)
        empty = (Blo + Bhi) == 0
        Blo[empty] = 1
        meta = RelMeta(Blo, Bhi, groups)

        # absolute start column of each (window, half) bucket
        colstart = np.zeros(NW * 2, np.int64)
        gwin = np.arange(NW)
        colstart[gwin * 2] = meta.lo_start[gwin % W]
        colstart[gwin * 2 + 1] = meta.hi_start[gwin % W]

        bstart = np.zeros(NW * 2 + 1, np.int64)
        np.cumsum(cnt, out=bstart[1:])
        j_local = np.arange(len(dst_s)) - bstart[key_s]
        core = (key_s // 2) // W
        col = colstart[key_s] + (j_local >> 7)
        lane = j_local & 127

        NB = meta.NB
        vidx = np.zeros((C, NB, 128), np.int64)     # [core, col, lane]
        eoff = np.zeros((C, 128, NB), np.float32)
        ew = np.zeros((C, 128, NB), np.float32)
        vidx[core, col, lane] = src_s - (src_s >= HI0) * HI0
        eoff[core, lane, col] = (dst_s & 127).astype(np.float32)
        ew[core, lane, col] = w_s.astype(np.float32)

        # idx16[c, 16*rep + i%16, i//16] = flat idx i (= col*128 + lane)
        flat = vidx.reshape(C, NB * 128)
        t16 = flat.reshape(C, NB * 8, 16).transpose(0, 2, 1).astype(np.int16)
        idx16 = np.tile(t16, (1, 8, 1))             # [C, 128, 8*NB]
        out.append((idx16, eoff, ew, meta))
    return out


# ----------------------------------------------------------------------------
# Device program
# ----------------------------------------------------------------------------

def build_program(cfg: Cfg, metas, for_sim=False, debug=False):
    D, Q, R, W, NP, NT, KT = cfg.D, cfg.Q, cfg.R, cfg.W, cfg.NP, cfg.NT, cfg.KT
    GDT = cfg.gdt
    groups = make_groups(cfg)
    NBs = [m.NB for m in metas]
    # per-(group, rel) column extents and gather piece capacity
    gcap = 1
    capb = 1
    for r in range(R):
        for (g0, nlo, nhi) in metas[r].g_span:
            gcap = max(gcap, nlo + nhi)
            for (s0, s1) in (split_cols(g0, g0 + nlo, cfg.SUBCAP)
                             + split_cols(g0 + nlo, g0 + nlo + nhi, cfg.SUBCAP)):
                capb = max(capb, s1 - s0)

    nc = bacc.Bacc("TRN2", target_bir_lowering=False,
                   debug=False, enable_asserts=for_sim, num_devices=cfg.C)

    # ---- DRAM I/O ----
    xt8 = nc.dram_tensor("xt8", [NT // KT, 128, KT * D], F32, kind="ExternalInput")
    fw = nc.dram_tensor("fw", [D, D], F32, kind="ExternalInput")
    cw = nc.dram_tensor("cw", [2 * R, D, D], F32, kind="ExternalInput")
    cbt = nc.dram_tensor("cbt", [D, 2 * R], F32, kind="ExternalInput")
    awl = nc.dram_tensor("awl", [2, D, Q], F32, kind="ExternalInput")
    aqt = nc.dram_tensor("aqt", [Q, 2], F32, kind="ExternalInput")
    wct = nc.dram_tensor("wct", [3 * D, D], F32, kind="ExternalInput")
    iot = nc.dram_tensor("iot", [128, 128], F32, kind="ExternalInput")
    idn = nc.dram_tensor("idn", [128, 128], F32, kind="ExternalInput")
    oidx = nc.dram_tensor("oidx", [128, W], I32, kind="ExternalInput")
    didx16 = [nc.dram_tensor(f"idx16_{r}", [128, 8 * NBs[r]], I16, kind="ExternalInput")
              for r in range(R)]
    deoff = [nc.dram_tensor(f"eoff{r}", [128, NBs[r]], F32, kind="ExternalInput")
             for r in range(R)]
    dew = [nc.dram_tensor(f"ew{r}", [128, NBs[r]], F32, kind="ExternalInput")
           for r in range(R)]

    h0rep = nc.dram_tensor("h0rep", [cfg.NPAD, D], GDT,
                           **(dict(kind="ExternalOutput") if debug else {}))
    tshard = nc.dram_tensor("tshard", [NP, D], GDT)
    tfull = nc.dram_tensor("tfull", [cfg.NPAD, D], GDT, addr_space="Shared")
    if debug:
        dbg_h0t = nc.dram_tensor("dbg_h0t", [D, NP], F32, kind="ExternalOutput")
        dbg_h1t = nc.dram_tensor("dbg_h1t", [D, NP], F32, kind="ExternalOutput")
        dbg_tt = nc.dram_tensor("dbg_tt", [D, NP], F32, kind="ExternalOutput")
        dbg_tshard = nc.dram_tensor("dbg_tshard", [NP, D], GDT, kind="ExternalOutput")

    ytt = nc.dram_tensor("ytt", [128, NP], F32, kind="ExternalOutput")
    attnt = nc.dram_tensor("attnt", [128, 3 * W], F32, kind="ExternalOutput")

    with tile.TileContext(nc) as tc:
        import contextlib
        with contextlib.ExitStack() as ctx:
            cpool = ctx.enter_context(tc.tile_pool(name="const", bufs=1))
            xpool = ctx.enter_context(tc.tile_pool(name="xin", bufs=3))
            h0wb = ctx.enter_context(tc.tile_pool(name="h0wb", bufs=3))
            gpool = ctx.enter_context(tc.tile_pool(name="gbuf", bufs=2))
            tpool = ctx.enter_context(tc.tile_pool(name="tabs", bufs=3))
            opool = ctx.enter_context(tc.tile_pool(name="ownr", bufs=3))
            spool = ctx.enter_context(tc.tile_pool(name="sel", bufs=6))
            apool = ctx.enter_context(tc.tile_pool(name="accT", bufs=3 * cfg.G_W + 2))
            dpool = ctx.enter_context(tc.tile_pool(name="dstream", bufs=3))
            psagg = ctx.enter_context(tc.tile_pool(name="psagg", bufs=3, space="PSUM"))
            psd = ctx.enter_context(tc.tile_pool(name="psd", bufs=4, space="PSUM"))
            psh = ctx.enter_context(tc.tile_pool(name="psh", bufs=1, space="PSUM"))

            # ---- constants into SBUF ----
            sb_fw = cpool.tile([D, D], F32, tag="fw")
            nc.sync.dma_start(out=sb_fw[:], in_=fw[:])
            sb_cw = []
            for j in range(2 * R):
                t = cpool.tile([D, D], F32, tag=f"cw{j}")
                nc.sync.dma_start(out=t[:], in_=cw[j])
                sb_cw.append(t)
            sb_cbt = cpool.tile([D, 2 * R], F32, tag="cbt")
            nc.sync.dma_start(out=sb_cbt[:], in_=cbt[:])
            sb_awl = []
            for l in range(2):
                t = cpool.tile([D, Q], F32, tag=f"awl{l}")
                nc.sync.dma_start(out=t[:], in_=awl[l])
                sb_awl.append(t)
            sb_aq = []
            for l in range(2):
                t = cpool.tile([Q, 1], F32, tag=f"aq{l}")
                nc.sync.dma_start(out=t[:], in_=aqt[:, l:l + 1])
                sb_aq.append(t)
            sb_wct = []
            for j in range(3):
                t = cpool.tile([D, D], F32, tag=f"wct{j}")
                nc.sync.dma_start(out=t[:], in_=wct[j * D:(j + 1) * D, :])
                sb_wct.append(t)
            sb_iota = cpool.tile([128, 128], F32, tag="iota")
            nc.sync.dma_start(out=sb_iota[:], in_=iot[:])
            sb_ident = cpool.tile([128, 128], F32, tag="ident")
            nc.sync.dma_start(out=sb_ident[:], in_=idn[:])
            sb_oidx = cpool.tile([128, W], I32, tag="oidx")
            nc.sync.dma_start(out=sb_oidx[:], in_=oidx[:])

            # persistent transposed own-shard slabs
            sb_h0T = cpool.tile([D, NP], F32, tag="h0T")
            sb_h1T = cpool.tile([D, NP], F32, tag="h1T")
            sb_tT = cpool.tile([D, NP], F32, tag="tT")
            sb_attn = cpool.tile([128, 3 * W], F32, tag="attnsl")

            # ---------------- h0 = tanh(x @ fw), replicated ----------------
            for gi in range(NT // KT):
                xt = xpool.tile([128, KT * D], F32, tag="xt")
                nc.sync.dma_start(out=xt[:], in_=xt8[gi])
                wide = h0wb.tile([128, KT * D], GDT, tag="h0w")
                for k in range(0, KT, 4):
                    ph = psh.tile([128, 4 * D], F32, tag="pd4")
                    nc.tensor.matmul(out=ph[:], lhsT=sb_fw[:],
                                     rhs=xt[:, k * D:(k + 4) * D],
                                     start=True, stop=True)
                    h0t4 = h0wb.tile([128, 4 * D], F32, tag="h0t4")
                    nc.scalar.activation(h0t4[:], ph[:], ACTF.Tanh)
                    for k2 in range(4):
                        ptt = psd.tile([128, D], F32, tag="pd")
                        nc.tensor.transpose(ptt[:], h0t4[:, k2 * D:(k2 + 1) * D], sb_ident[:])
                        nc.vector.tensor_copy(wide[:, (k + k2) * D:(k + k2 + 1) * D], ptt[:])
                dst = h0rep[gi * KT * 128:(gi + 1) * KT * 128, :]
                dst = dst.rearrange("(k p) d -> p k d", p=128)
                src3 = wide[:].rearrange("p (k d) -> p k d", k=KT)
                nc.sync.dma_start(out=dst, in_=src3)

            # own-shard rows -> transposed slab (per-partition indirect DMA,
            # one window = 128 rows per instruction)
            def fill_own_slab(slab, src):
                for wl in range(W):
                    ot = opool.tile([128, D], GDT, tag="own")
                    nc.gpsimd.indirect_dma_start(
                        out=ot[:], out_offset=None, in_=src[:],
                        in_offset=IndirectOffsetOnAxis(ap=sb_oidx[:, wl:wl + 1], axis=0))
                    ptt = psd.tile([128, D], F32, tag="pd")
                    if GDT != F32:
                        o32 = opool.tile([128, D], F32, tag="own32")
                        nc.vector.tensor_copy(o32[:], ot[:])
                        nc.tensor.transpose(ptt[:], o32[:], sb_ident[:])
                    else:
                        nc.tensor.transpose(ptt[:], ot[:], sb_ident[:])
                    nc.scalar.copy(slab[:, wl * 128:(wl + 1) * 128], ptt[:])

            fill_own_slab(sb_h0T, h0rep)

            # ---------------- shared layer emitter ----------------
            def emit_layer(kind: str):
                lw = 0 if kind == "B" else 1
                src = tfull if kind == "C" else h0rep
                resid = sb_tT if kind == "C" else sb_h0T
                src_hi = src[cfg.HI0:, :] if cfg.NPAD > cfg.HI0 else None
                for gi, (w0, nw) in enumerate(groups):
                    gtiles = []   # (r, s0, s1, tile)
                    taboff = []   # (g0, toff, tw) per r
                    for r in range(R):
                        meta = metas[r]
                        g0, nlo, nhi = meta.g_span[gi]
                        ncols = nlo + nhi
                        toff = tpool.tile([128, gcap], F32, tag="toff")
                        nc.sync.dma_start(out=toff[:, :ncols],
                                          in_=deoff[r][:, g0:g0 + ncols])
                        tw = tpool.tile([128, gcap], F32, tag="tw")
                        nc.sync.dma_start(out=tw[:, :ncols],
                                          in_=dew[r][:, g0:g0 + ncols])
                        tix = tpool.tile([128, 8 * gcap], I16, tag="tix")
                        nc.sync.dma_start(out=tix[:, :8 * ncols],
                                          in_=didx16[r][:, 8 * g0:8 * (g0 + ncols)])
                        taboff.append((g0, toff, tw))
                        for half, h0_, h1_ in (("lo", g0, g0 + nlo),
                                               ("hi", g0 + nlo, g0 + nlo + nhi)):
                            base = src if half == "lo" else src_hi
                            for (s0, s1) in split_cols(h0_, h1_, cfg.SUBCAP):
                                gt = gpool.tile([128, capb * D], GDT, tag="gbuf")
                                nblk = s1 - s0
                                out3 = gt[:].rearrange("p (b e) -> p b e", e=D)
                                nc.gpsimd.dma_gather(
                                    out3[:, :nblk, :], base[:],
                                    tix[:, 8 * (s0 - g0):8 * (s1 - g0)],
                                    nblk * 128, nblk * 128, D)
                                gtiles.append((r, s0, s1, gt))

                    def gslice(r, col):
                        for (rr, s0, s1, gt) in gtiles:
                            if rr == r and s0 <= col < s1:
                                lc = col - s0
                                return gt[:, lc * D:(lc + 1) * D]
                        raise AssertionError

                    accT = {}
                    for r in range(R):
                        meta = metas[r]
                        g0, toff, tw = taboff[r]
                        for wl in range(w0, w0 + nw):
                            pagg = psagg.tile([128, D], F32, tag="agg")
                            cols = meta.win_cols(wl)
                            for k, col in enumerate(cols):
                                lc = col - g0
                                S = spool.tile([128, 128], GDT, tag="sel")
                                nc.vector.tensor_scalar(
                                    S[:], sb_iota[:],
                                    toff[:, lc:lc + 1], tw[:, lc:lc + 1],
                                    ALU.is_equal, ALU.mult)
                                nc.tensor.matmul(out=pagg[:], lhsT=gslice(r, col),
                                                 rhs=S[:], start=(k == 0),
                                                 stop=(k == len(cols) - 1))
                            at = apool.tile([128, D], F32, tag="accT")
                            nc.scalar.copy(at[:], pagg[:])
                            accT[(r, wl)] = at

                    # downstream per window
                    for wl in range(w0, w0 + nw):
                        wsl = slice(wl * 128, (wl + 1) * 128)
                        vT = []
                        for r in range(R):
                            lr = 3 * lw + r
                            pv = psd.tile([128, D], F32, tag="pd")
                            nc.tensor.matmul(out=pv[:], lhsT=sb_cw[lr][:],
                                             rhs=accT[(r, wl)][:],
                                             start=True, stop=True)
                            v0 = dpool.tile([128, D], F32, tag="v0")
                            nc.scalar.activation(v0[:], pv[:], ACTF.Identity,
                                                 bias=sb_cbt[:, lr:lr + 1])
                            vt = dpool.tile([128, D], F32, tag=f"vt{r}")
                            nc.vector.tensor_tensor(vt[:], v0[:], resid[:, wsl], ALU.add)
                            vT.append(vt)
                        es = []
                        for r in range(R):
                            pk = psd.tile([Q, 128], F32, tag="pd")
                            nc.tensor.matmul(out=pk[:], lhsT=sb_awl[lw][:],
                                             rhs=vT[r][:], start=True, stop=True)
                            kt = dpool.tile([Q, 128], F32, tag="kt")
                            nc.scalar.activation(kt[:], pk[:], ACTF.Tanh)
                            ps = psd.tile([128, 1], F32, tag="pd")
                            nc.tensor.matmul(out=ps[:], lhsT=kt[:], rhs=sb_aq[lw][:],
                                             start=True, stop=True)
                            e = dpool.tile([128, 1], F32, tag=f"e{r}")
                            nc.scalar.activation(e[:], ps[:], ACTF.Exp)
                            es.append(e)
                        ssum = dpool.tile([128, 1], F32, tag="ssum")
                        nc.vector.tensor_tensor(ssum[:], es[0][:], es[1][:], ALU.add)
                        nc.vector.tensor_tensor(ssum[:], ssum[:], es[2][:], ALU.add)
                        rinv = dpool.tile([128, 1], F32, tag="rinv")
                        nc.vector.reciprocal(rinv[:], ssum[:])
                        ar = []
                        for r in range(R):
                            if kind == "C":
                                a = sb_attn[:, 3 * wl + r:3 * wl + r + 1]
                            else:
                                at_ = dpool.tile([128, 1], F32, tag=f"a{r}")
                                a = at_[:]
                            nc.vector.tensor_tensor(a, es[r][:], rinv[:], ALU.mult)
                            ar.append(a)
                        comb = None
                        for r in range(R):
                            pvt = psd.tile([128, D], F32, tag="pd")
                            nc.tensor.transpose(pvt[:], vT[r][:], sb_ident[:])
                            m = dpool.tile([128, D], F32, tag="cm")
                            nc.vector.tensor_scalar(m[:], pvt[:], ar[r], None, ALU.mult)
                            if comb is None:
                                comb = m
                            else:
                                c2 = dpool.tile([128, D], F32, tag=f"cb{r}")
                                nc.vector.tensor_tensor(c2[:], comb[:], m[:], ALU.add)
                                comb = c2

                        if kind == "B":
                            if GDT != F32:
                                cb = dpool.tile([128, D], GDT, tag="cbf")
                                nc.vector.tensor_copy(cb[:], comb[:])
                                nc.sync.dma_start(out=tshard[wsl, :], in_=cb[:])
                            else:
                                nc.sync.dma_start(out=tshard[wsl, :], in_=comb[:])
                        elif kind == "A":
                            ptt = psd.tile([128, D], F32, tag="pd")
                            nc.tensor.transpose(ptt[:], comb[:], sb_ident[:])
                            nc.scalar.copy(sb_h1T[:, wsl], ptt[:])
                        else:  # C: final projection
                            ptt = psd.tile([128, D], F32, tag="pd")
                            nc.tensor.transpose(ptt[:], comb[:], sb_ident[:])
                            h2t = dpool.tile([128, D], F32, tag="h2t")
                            nc.scalar.copy(h2t[:], ptt[:])
                            py = psd.tile([128, D], F32, tag="pd")
                            nc.tensor.matmul(out=py[:], lhsT=sb_wct[0][:],
                                             rhs=sb_h0T[:, wsl], start=True, stop=False)
                            nc.tensor.matmul(out=py[:], lhsT=sb_wct[1][:],
                                             rhs=sb_h1T[:, wsl], start=False, stop=False)
                            nc.tensor.matmul(out=py[:], lhsT=sb_wct[2][:],
                                             rhs=h2t[:], start=False, stop=True)
                            ysb = dpool.tile([128, D], F32, tag="ysb")
                            nc.vector.tensor_copy(ysb[:], py[:])
                            nc.sync.dma_start(out=ytt[:, wsl], in_=ysb[:])

            # ---------------- layers ----------------
            emit_layer("B")      # t = het(h0, l=0) -> tshard
            nc.gpsimd.collective_compute(
                "AllGather", ALU.bypass,
                replica_groups=[list(range(cfg.C))],
                ins=[tshard.ap().opt()], outs=[tfull.ap().opt()])
            emit_layer("A")      # h1 = het(h0, l=1) -> sb_h1T (overlaps collective)
            fill_own_slab(sb_tT, tfull)
            emit_layer("C")      # h2/attn/y

            nc.sync.dma_start(out=attnt[:], in_=sb_attn[:])
            if debug:
                nc.sync.dma_start(out=dbg_h0t[:], in_=sb_h0T[:])
                nc.sync.dma_start(out=dbg_h1t[:], in_=sb_h1T[:])
                nc.sync.dma_start(out=dbg_tt[:], in_=sb_tT[:])
                for i0 in range(0, NP, 128 * 32):
                    i1 = min(i0 + 128 * 32, NP)
                    bt = gpool.tile([128, 32 * D], GDT, tag="dbgb")
                    n32 = (i1 - i0) // 128
                    bsrc = tshard[i0:i1, :].rearrange("(k p) d -> p k d", p=128)
                    nc.sync.dma_start(
                        out=bt[:].rearrange("p (k d) -> p k d", k=32)[:, :n32, :],
                        in_=bsrc)
                    bdst = dbg_tshard[i0:i1, :].rearrange("(k p) d -> p k d", p=128)
                    nc.sync.dma_start(
                        out=bdst,
                        in_=bt[:].rearrange("p (k d) -> p k d", k=32)[:, :n32, :])

    nc.compile()
    return nc


# ----------------------------------------------------------------------------
# Host wrapper
# ----------------------------------------------------------------------------

def make_host_data(inputs, cfg: Cfg):
    x = np.asarray(inputs["x"], np.float32)
    edges = np.asarray(inputs["edges"])
    feat_w = np.asarray(inputs["feat_w"], np.float32)
    conv_w = np.asarray(inputs["conv_w"], np.float32)
    conv_b = np.asarray(inputs["conv_b"], np.float32)
    attn_w = np.asarray(inputs["attn_w"], np.float32)
    attn_q = np.asarray(inputs["attn_q"], np.float32)
    concat_w = np.asarray(inputs["concat_w"], np.float32)

    D, KT, NT = cfg.D, cfg.KT, cfg.NT
    xpad = np.zeros((cfg.NPAD, D), np.float32)
    xpad[:cfg.N] = x
    xt8 = xpad.reshape(NT // KT, KT, 128, D).transpose(0, 3, 1, 2)
    xt8 = np.ascontiguousarray(xt8.reshape(NT // KT, D, KT * 128))

    tabs = preprocess_edges(edges, cfg)

    shared = {
        "xt8": xt8,
        "fw": np.ascontiguousarray(feat_w),
        "cw": np.ascontiguousarray(conv_w.reshape(2 * cfg.R, D, D)),
        "cbt": np.ascontiguousarray(conv_b.reshape(2 * cfg.R, D).T),
        "awl": np.ascontiguousarray(attn_w),
        "aqt": np.ascontiguousarray(attn_q.T),
        "wct": np.ascontiguousarray(concat_w.T),
        "iot": np.broadcast_to(np.arange(128, dtype=np.float32), (128, 128)).copy(),
        "idn": np.eye(128, dtype=np.float32),
    }
    in_maps = []
    for c in range(cfg.C):
        m = dict(shared)
        p = np.arange(128, dtype=np.int32)[:, None]
        j = np.arange(cfg.W, dtype=np.int32)[None, :]
        m["oidx"] = np.ascontiguousarray(c * cfg.NP + j * 128 + p)
        for r in range(cfg.R):
            idx16, eoff, ew, _ = tabs[r]
            m[f"idx16_{r}"] = np.ascontiguousarray(idx16[c])
            m[f"eoff{r}"] = np.ascontiguousarray(eoff[c])
            m[f"ew{r}"] = np.ascontiguousarray(ew[c])
        in_maps.append(m)
    metas = [tabs[r][3] for r in range(cfg.R)]
    return in_maps, metas


def postprocess(results, cfg: Cfg):
    ys, attns = [], []
    for c in range(cfg.C):
        ys.append(np.asarray(results[c]["ytt"]).T)            # [NP, 128]
        sl = np.asarray(results[c]["attnt"]).reshape(128, cfg.W, 3)
        attns.append(sl.transpose(1, 0, 2).reshape(cfg.NP, 3))
    y = np.concatenate(ys, axis=0)[:cfg.N]
    attn = np.concatenate(attns, axis=0)[:cfg.N]
    return np.ascontiguousarray(y), np.ascontiguousarray(attn)


_PROG_CACHE: dict = {}


def _get_program(cfg: Cfg, metas):
    key = (cfg, tuple(m.NB for m in metas),
           tuple(tuple(int(v) for v in m.Blo) + tuple(int(v) for v in m.Bhi)
                 for m in metas))
    if key not in _PROG_CACHE:
        _PROG_CACHE[key] = build_program(cfg, metas)
    return _PROG_CACHE[key]


def _install_ntff_hook():
    if "antenv.axon_hooks" in sys.modules:
        return
    try:
        from trn_agent_boot.trn_boot import _ntff_profile_via_ctypes
        hook = _ntff_profile_via_ctypes("/opt/axon/libaxon_pjrt.so")
    except Exception:
        hook = None
    mod = types.ModuleType("antenv.axon_hooks")
    mod.get_axon_ntff_profile_hook = lambda: hook
    mod.set_axon_ntff_profile_hook = lambda h: None
    sys.modules["antenv.axon_hooks"] = mod


def kernel_impl(inputs, cfg: Cfg, trace=False):
    in_maps, metas = make_host_data(inputs, cfg)
    nc = _get_program(cfg, metas)
    if trace:
        _install_ntff_hook()
    res = run_bass_kernel_spmd(nc, in_maps, list(range(cfg.C)), trace=trace)
    y, attn = postprocess(res.results, cfg)
    return (y, attn), res.exec_time_ns


def kernel(**inputs):
    (y, attn), _ = kernel_impl(inputs, FULL_CFG)
    return y, attn
